# revision 20
# baseline (speedup 1.0000x reference)
"""Trainium2 kernel for nn_LoRALinear (moe_routing).

Math: reference computes out = x @ W.T + einsum('bri,bro->bo', a, b) with
a = A_table[dom].reshape(B,R,IN), b = B_table[dom].reshape(B,R,OUT).
The einsum contracts i over `a` alone, so the LoRA term collapses to a
per-domain table:
    L[d, o] = sum_r (sum_i A_table[d].reshape(R,IN)[r,i]) * B_table[d].reshape(R,OUT)[r,o]
    out = x @ W.T + L[domain_id]

On device: pure dense matmul x @ W.T (K=1024 as 8 chunks of 128) in bf16;
the LoRA term is added during PSUM evacuation by the vector engine from a
host-gathered per-row table Lg = L[domain_id] streamed in as bf16.
Output is written in bf16 (rel-err budget 2e-2; bf16 rounding adds ~1e-3)
and upcast to f32 on the host.

Sharding: data-parallel over batch across 8 cores; W replicated.

Scheduling notes (from trace iterations):
- Every dma_start costs ~0.6us dispatch time on its HWDGE ring and the
  rings drain FIFO, so all input loads ride the sync ring in exact need
  order: x-block-0 pair chunks interleaved with W chunks, lg block 0 (in
  per-m-tile pieces), then (x, lg) per later block.  Any early load on
  the other ring steals HBM wire time from the W stream and starves the
  prologue.  Out stores ride the ACT ring.
- 36 small warmup matmuls keep the PE busy from preamble-exit (~7us)
  until the first W chunk lands (~11us) so the HAM clock gate is open
  (2.4 GHz) when real work starts.
- Prologue covers block-0 m-tiles 0-2 k-major; m-tile 2 joins at chunk 2
  (accumulation order rotated) so early consumption matches the
  receipt-latency-bound supply of the first W chunks.
- The last m-tile's two halves store separately on the two rings to
  shorten the drain tail.
"""

import functools

import numpy as np

import concourse.mybir as mybir
import concourse.tile as tile
from concourse import bacc, bass_utils

B, D, R, ND = 16384, 1024, 8, 64
N_CORES = 8
BS = B // N_CORES            # 2048 batch rows per core
NK = 8                       # K chunks of 128
MB = 512                     # batch rows per xa block (4 m-tiles)
NMB = BS // MB               # 4 blocks
NMT = 4                      # m-tiles per block
OH = 512                     # psum free dim (one bank)


@functools.lru_cache(maxsize=1)
def _build():
    nc = bacc.Bacc(None, target_bir_lowering=False, debug=False)
    bf16 = mybir.dt.bfloat16
    f32 = mybir.dt.float32
    xa = nc.dram_tensor("xa", [128, NMB * NK * MB], bf16, kind="ExternalInput")
    wa = nc.dram_tensor("wa", [NK * 128, D], bf16, kind="ExternalInput")
    # Lg rows packed per m-tile: lga[p, mb, mt, o] = L[dom[(mb*4+mt)*128+p], o]
    lga = nc.dram_tensor("lga", [128, NMB * NMT * D], bf16, kind="ExternalInput")
    out = nc.dram_tensor("out", [BS, D], bf16, kind="ExternalOutput")

    with tile.TileContext(nc) as tc:
        with (
            tc.tile_pool(name="w", bufs=1) as wpool,
            tc.tile_pool(name="x", bufs=4) as xpool,
            tc.tile_pool(name="lg", bufs=4) as lgpool,
            tc.tile_pool(name="o", bufs=6) as opool,
            tc.tile_pool(name="ps", bufs=8, space="PSUM") as pspool,
        ):
            # --- input loads, sync ring, strict need order.
            xts = {}
            xt0 = xpool.tile([128, NK * MB], bf16, tag="x")
            wts = [None] * NK
            for p in range(4):
                nc.sync.dma_start(
                    xt0[:, 2 * p * MB : 2 * (p + 1) * MB],
                    xa[:, 2 * p * MB : 2 * (p + 1) * MB],
                )
                for k in (2 * p, 2 * p + 1):
                    wt = wpool.tile([128, D], bf16, tag=f"w{k}", name=f"wt{k}")
                    nc.sync.dma_start(wt[:], wa[k * 128 : (k + 1) * 128, :])
                    wts[k] = wt
            xts[0] = xt0
            # lg block 0 in per-m-tile pieces (first finish isn't gated on
            # the whole block landing).
            lgts = {}
            lgt0 = lgpool.tile([128, NMT * D], bf16, tag="lg")
            for mt in range(NMT):
                nc.sync.dma_start(
                    lgt0[:, mt * D : (mt + 1) * D],
                    lga[:, mt * D : (mt + 1) * D],
                )
            lgts[0] = lgt0
            # Later x and lg blocks, interleaved in need order.
            for mb in range(1, NMB):
                xtn = xpool.tile([128, NK * MB], bf16, tag="x", name=f"xt{mb}")
                nc.sync.dma_start(xtn[:], xa[:, mb * NK * MB : (mb + 1) * NK * MB])
                xts[mb] = xtn
                lgtn = lgpool.tile([128, NMT * D], bf16, tag="lg", name=f"lgt{mb}")
                nc.sync.dma_start(lgtn[:], lga[:, mb * NMT * D : (mb + 1) * NMT * D])
                lgts[mb] = lgtn

            # --- PE warmup (HAM clock gate) until the first W chunk lands.
            scratch = wpool.tile([128, 256], bf16, tag="scratch")
            nc.gpsimd.memset(scratch[:], 0.0)
            psw = pspool.tile([128, OH], f32, tag="ps")
            for i in range(36):
                nc.tensor.matmul(
                    psw[:, 0:128],
                    scratch[:, 0:128],
                    scratch[:, 128:256],
                    start=(i == 0),
                    stop=(i == 35),
                )

            def xsl(xt, k, mt):
                return xt[:, k * MB + mt * 128 : k * MB + (mt + 1) * 128]

            def finish(mb, mt, ps0, ps1, lgt):
                ot = opool.tile([128, D], bf16, tag="ot")
                nc.vector.tensor_add(ot[:, 0:OH], ps0[:], lgt[:, mt * D : mt * D + OH])
                nc.vector.tensor_add(ot[:, OH:D], ps1[:], lgt[:, mt * D + OH : (mt + 1) * D])
                m0 = (mb * NMT + mt) * 128
                nc.scalar.dma_start(out[m0 : m0 + 128, :], ot[:])

            # --- prologue: block 0, m-tiles 0..2, k-major across 6 psum
            # banks.  m-tile 2 joins at chunk 2 (its accumulation order is
            # rotated) so chunk-0/1 consumption (4 matmuls each) matches
            # the receipt-bound early supply; its last chunks run after
            # chunk 7, when supply is no longer binding.
            pss = [
                [
                    pspool.tile([128, OH], f32, tag="ps", name=f"ps_{mt}_{h}")
                    for h in range(2)
                ]
                for mt in range(3)
            ]
            for k in range(NK):
                for mt in range(3):
                    if mt == 2 and k < 2:
                        continue
                    for h in range(2):
                        nc.tensor.matmul(
                            pss[mt][h][:],
                            xsl(xt0, k, mt),
                            wts[k][:, h * OH : (h + 1) * OH],
                            start=(k == 0 or (mt == 2 and k == 2)),
                            stop=(k == NK - 1 and mt < 2),
                        )
            for k in (0, 1):
                for h in range(2):
                    nc.tensor.matmul(
                        pss[2][h][:],
                        xsl(xt0, k, 2),
                        wts[k][:, h * OH : (h + 1) * OH],
                        start=False,
                        stop=(k == 1),
                    )
            for mt in range(3):
                finish(0, mt, pss[mt][0], pss[mt][1], lgt0)

            # --- steady state: remaining m-tiles, halves-outer so ps0
            # stops (and evacuates) 8 matmuls before ps1.
            tiles = [(0, 3)] + [(mb, mt) for mb in range(1, NMB) for mt in range(NMT)]
            for mb, mt in tiles:
                last = (mb, mt) == tiles[-1]
                xt = xts[mb]
                ps0 = pspool.tile([128, OH], f32, tag="ps", name="ps0")
                ps1 = pspool.tile([128, OH], f32, tag="ps", name="ps1")
                for h, ps in ((0, ps0), (1, ps1)):
                    for k in range(NK):
                        nc.tensor.matmul(
                            ps[:], xsl(xt, k, mt), wts[k][:, h * OH : (h + 1) * OH],
                            start=(k == 0), stop=(k == NK - 1),
                        )
                lgt = lgts[mb]
                if not last:
                    finish(mb, mt, ps0, ps1, lgt)
                else:
                    # split tail: half 0 stores via the (idle) sync ring as
                    # soon as ps0 is evacuated; half 1 via the ACT ring.
                    m0 = (mb * NMT + mt) * 128
                    ot = opool.tile([128, D], bf16, tag="ot")
                    nc.vector.tensor_add(ot[:, 0:OH], ps0[:], lgt[:, mt * D : mt * D + OH])
                    nc.sync.dma_start(out[m0 : m0 + 128, 0:OH], ot[:, 0:OH])
                    nc.vector.tensor_add(ot[:, OH:D], ps1[:], lgt[:, mt * D + OH : (mt + 1) * D])
                    nc.scalar.dma_start(out[m0 : m0 + 128, OH:D], ot[:, OH:D])

    nc.compile()
    return nc


def _prepare(x, W, A_table, B_table, domain_id):
    import ml_dtypes

    bf16 = np.dtype(ml_dtypes.bfloat16)
    x = np.asarray(x, dtype=np.float32)
    W = np.asarray(W, dtype=np.float32)
    A = np.asarray(A_table, dtype=np.float64)
    Bt = np.asarray(B_table, dtype=np.float64)
    dom = np.asarray(domain_id).astype(np.int64)

    sA = A.reshape(ND, R, D).sum(axis=2)                        # [ND, R]
    L = np.einsum("dr,dro->do", sA, Bt.reshape(ND, R, D))       # [ND, D]
    Lg = L.astype(np.float32)[dom].astype(bf16)                 # [B, D]

    wa = np.ascontiguousarray(W.T.astype(bf16))                 # [D, D]
    xT = np.ascontiguousarray(x.T).astype(bf16)                 # [D, B]

    in_maps = []
    for c in range(N_CORES):
        sl = slice(c * BS, (c + 1) * BS)
        # chunk-major: xa[p, mb, k, j] = xT[k*128 + p, c*BS + mb*MB + j]
        xa_c = np.ascontiguousarray(
            xT[:, sl].reshape(NK, 128, NMB, MB).transpose(1, 2, 0, 3)
        ).reshape(128, NMB * NK * MB)
        # lga[p, mb, mt, o] = Lg[c*BS + (mb*4+mt)*128 + p, o]
        lga_c = np.ascontiguousarray(
            Lg[sl].reshape(NMB, NMT, 128, D).transpose(2, 0, 1, 3)
        ).reshape(128, NMB * NMT * D)
        in_maps.append({"xa": xa_c, "wa": wa, "lga": lga_c})
    return in_maps


def kernel(x, W, A_table, B_table, domain_id, _trace=False):
    in_maps = _prepare(x, W, A_table, B_table, domain_id)
    nc = _build()
    res = bass_utils.run_bass_kernel_spmd(
        nc, in_maps, core_ids=list(range(N_CORES)), trace=_trace
    )
    out = np.concatenate(
        [np.asarray(res.results[c]["out"]).astype(np.float32) for c in range(N_CORES)],
        axis=0,
    )
    if _trace:
        kernel.last_results = res
    return out


# revision 22
# speedup vs baseline: 1.0027x; 1.0027x over previous
"""Trainium2 kernel for nn_LoRALinear (moe_routing).

Math: reference computes out = x @ W.T + einsum('bri,bro->bo', a, b) with
a = A_table[dom].reshape(B,R,IN), b = B_table[dom].reshape(B,R,OUT).
The einsum contracts i over `a` alone, so the LoRA term collapses to a
per-domain table:
    L[d, o] = sum_r (sum_i A_table[d].reshape(R,IN)[r,i]) * B_table[d].reshape(R,OUT)[r,o]
    out = x @ W.T + L[domain_id]

On device: pure dense matmul x @ W.T (K=1024 as 8 chunks of 128) in bf16;
the LoRA term is added during PSUM evacuation by the vector engine from a
host-gathered per-row table Lg = L[domain_id] streamed in as bf16.
Output is written in bf16 (rel-err budget 2e-2; bf16 rounding adds ~1e-3)
and upcast to f32 on the host.

Sharding: data-parallel over batch across 8 cores; W replicated.

Scheduling notes (from trace iterations):
- Every dma_start costs ~0.6us dispatch time on its HWDGE ring and the
  rings drain FIFO, so all input loads ride the sync ring in exact need
  order: x-block-0 pair chunks interleaved with W chunks, lg block 0 (in
  per-m-tile pieces), then (x, lg) per later block.  Any early load on
  the other ring steals HBM wire time from the W stream and starves the
  prologue.  Out stores ride the ACT ring.
- 36 small warmup matmuls keep the PE busy from preamble-exit (~7us)
  until the first W chunk lands (~11us) so the HAM clock gate is open
  (2.4 GHz) when real work starts.
- Prologue covers block-0 m-tiles 0-2 k-major; m-tile 2 joins at chunk 2
  (accumulation order rotated) so early consumption matches the
  receipt-latency-bound supply of the first W chunks.
- The last m-tile's two halves store separately on the two rings to
  shorten the drain tail.
"""

import functools

import numpy as np

import concourse.mybir as mybir
import concourse.tile as tile
from concourse import bacc, bass_utils

B, D, R, ND = 16384, 1024, 8, 64
N_CORES = 8
BS = B // N_CORES            # 2048 batch rows per core
NK = 8                       # K chunks of 128
MB = 512                     # batch rows per xa block (4 m-tiles)
NMB = BS // MB               # 4 blocks
NMT = 4                      # m-tiles per block
OH = 512                     # psum free dim (one bank)


@functools.lru_cache(maxsize=1)
def _build():
    nc = bacc.Bacc(None, target_bir_lowering=False, debug=False)
    bf16 = mybir.dt.bfloat16
    f32 = mybir.dt.float32
    xa = nc.dram_tensor("xa", [128, NMB * NK * MB], bf16, kind="ExternalInput")
    wa = nc.dram_tensor("wa", [NK * 128, D], bf16, kind="ExternalInput")
    # Lg rows packed per m-tile: lga[p, mb, mt, o] = L[dom[(mb*4+mt)*128+p], o]
    lga = nc.dram_tensor("lga", [128, NMB * NMT * D], bf16, kind="ExternalInput")
    out = nc.dram_tensor("out", [BS, D], bf16, kind="ExternalOutput")

    with tile.TileContext(nc) as tc:
        with (
            tc.tile_pool(name="w", bufs=1) as wpool,
            tc.tile_pool(name="x", bufs=4) as xpool,
            tc.tile_pool(name="lg", bufs=4) as lgpool,
            tc.tile_pool(name="o", bufs=6) as opool,
            tc.tile_pool(name="ps", bufs=8, space="PSUM") as pspool,
        ):
            # --- input loads, sync ring, strict need order.
            xts = {}
            xt0 = xpool.tile([128, NK * MB], bf16, tag="x")
            wts = [None] * NK
            for p in range(4):
                nc.sync.dma_start(
                    xt0[:, 2 * p * MB : 2 * (p + 1) * MB],
                    xa[:, 2 * p * MB : 2 * (p + 1) * MB],
                )
                for k in (2 * p, 2 * p + 1):
                    wt = wpool.tile([128, D], bf16, tag=f"w{k}", name=f"wt{k}")
                    nc.sync.dma_start(wt[:], wa[k * 128 : (k + 1) * 128, :])
                    wts[k] = wt
            xts[0] = xt0
            # lg block 0 in per-m-tile pieces (first finish isn't gated on
            # the whole block landing).
            lgts = {}
            lgt0 = lgpool.tile([128, NMT * D], bf16, tag="lg")
            for mt in range(NMT):
                nc.sync.dma_start(
                    lgt0[:, mt * D : (mt + 1) * D],
                    lga[:, mt * D : (mt + 1) * D],
                )
            lgts[0] = lgt0
            # Later x and lg blocks, interleaved in need order.
            for mb in range(1, NMB):
                xtn = xpool.tile([128, NK * MB], bf16, tag="x", name=f"xt{mb}")
                nc.sync.dma_start(xtn[:], xa[:, mb * NK * MB : (mb + 1) * NK * MB])
                xts[mb] = xtn
                lgtn = lgpool.tile([128, NMT * D], bf16, tag="lg", name=f"lgt{mb}")
                nc.sync.dma_start(lgtn[:], lga[:, mb * NMT * D : (mb + 1) * NMT * D])
                lgts[mb] = lgtn

            # --- PE warmup (HAM clock gate) until the first W chunk lands.
            scratch = wpool.tile([128, 256], bf16, tag="scratch")
            nc.gpsimd.memset(scratch[:], 0.0)
            psw = pspool.tile([128, OH], f32, tag="ps")
            for i in range(36):
                nc.tensor.matmul(
                    psw[:, 0:128],
                    scratch[:, 0:128],
                    scratch[:, 128:256],
                    start=(i == 0),
                    stop=(i == 35),
                )

            def xsl(xt, k, mt):
                return xt[:, k * MB + mt * 128 : k * MB + (mt + 1) * 128]

            ring = [nc.sync, nc.scalar]

            def finish(mb, mt, ps0, ps1, lgt):
                ot = opool.tile([128, D], bf16, tag="ot")
                nc.vector.tensor_add(ot[:, 0:OH], ps0[:], lgt[:, mt * D : mt * D + OH])
                nc.vector.tensor_add(ot[:, OH:D], ps1[:], lgt[:, mt * D + OH : (mt + 1) * D])
                m0 = (mb * NMT + mt) * 128
                nc.scalar.dma_start(out[m0 : m0 + 128, :], ot[:])

            # --- prologue: block 0, m-tiles 0..2, k-major across 6 psum
            # banks.  m-tile 2 joins at chunk 2 (its accumulation order is
            # rotated) so chunk-0/1 consumption (4 matmuls each) matches
            # the receipt-bound early supply; its last chunks run after
            # chunk 7, when supply is no longer binding.
            pss = [
                [
                    pspool.tile([128, OH], f32, tag="ps", name=f"ps_{mt}_{h}")
                    for h in range(2)
                ]
                for mt in range(3)
            ]
            for k in range(NK):
                for mt in range(3):
                    if mt == 2 and k < 2:
                        continue
                    for h in range(2):
                        nc.tensor.matmul(
                            pss[mt][h][:],
                            xsl(xt0, k, mt),
                            wts[k][:, h * OH : (h + 1) * OH],
                            start=(k == 0 or (mt == 2 and k == 2)),
                            stop=(k == NK - 1 and mt < 2),
                        )
            for k in (0, 1):
                for h in range(2):
                    nc.tensor.matmul(
                        pss[2][h][:],
                        xsl(xt0, k, 2),
                        wts[k][:, h * OH : (h + 1) * OH],
                        start=False,
                        stop=(k == 1),
                    )
            for mt in range(3):
                finish(0, mt, pss[mt][0], pss[mt][1], lgt0)

            # --- steady state: remaining m-tiles, halves-outer so ps0
            # stops (and evacuates) 8 matmuls before ps1.
            tiles = [(0, 3)] + [(mb, mt) for mb in range(1, NMB) for mt in range(NMT)]
            for mb, mt in tiles:
                last = (mb, mt) == tiles[-1]
                xt = xts[mb]
                lgt = lgts[mb]
                if not last:
                    ps0 = pspool.tile([128, OH], f32, tag="ps", name="ps0")
                    ps1 = pspool.tile([128, OH], f32, tag="ps", name="ps1")
                    for h, ps in ((0, ps0), (1, ps1)):
                        for k in range(NK):
                            nc.tensor.matmul(
                                ps[:], xsl(xt, k, mt), wts[k][:, h * OH : (h + 1) * OH],
                                start=(k == 0), stop=(k == NK - 1),
                            )
                    finish(mb, mt, ps0, ps1, lgt)
                else:
                    # quarter-granular tail: four N=256 accumulation groups
                    # (two per psum bank) that stop, evacuate, and store
                    # 64KB pieces on alternating rings, shortening the
                    # post-last-matmul drain chain.
                    m0 = (mb * NMT + mt) * 128
                    QH = OH // 2
                    ps0 = pspool.tile([128, OH], f32, tag="ps", name="ps0")
                    ps1 = pspool.tile([128, OH], f32, tag="ps", name="ps1")
                    for q in range(4):
                        ps = (ps0, ps1)[q // 2]
                        qs = (q % 2) * QH
                        c0 = q * QH
                        for k in range(NK):
                            nc.tensor.matmul(
                                ps[:, qs : qs + QH],
                                xsl(xt, k, mt),
                                wts[k][:, c0 : c0 + QH],
                                start=(k == 0), stop=(k == NK - 1),
                            )
                        otq = opool.tile([128, QH], bf16, tag="ot", name=f"otq{q}")
                        nc.vector.tensor_add(
                            otq[:], ps[:, qs : qs + QH],
                            lgt[:, mt * D + c0 : mt * D + c0 + QH],
                        )
                        ring[q % 2].dma_start(
                            out[m0 : m0 + 128, c0 : c0 + QH], otq[:]
                        )

    nc.compile()
    return nc


def _prepare(x, W, A_table, B_table, domain_id):
    import ml_dtypes

    bf16 = np.dtype(ml_dtypes.bfloat16)
    x = np.asarray(x, dtype=np.float32)
    W = np.asarray(W, dtype=np.float32)
    A = np.asarray(A_table, dtype=np.float64)
    Bt = np.asarray(B_table, dtype=np.float64)
    dom = np.asarray(domain_id).astype(np.int64)

    sA = A.reshape(ND, R, D).sum(axis=2)                        # [ND, R]
    L = np.einsum("dr,dro->do", sA, Bt.reshape(ND, R, D))       # [ND, D]
    Lg = L.astype(np.float32)[dom].astype(bf16)                 # [B, D]

    wa = np.ascontiguousarray(W.T.astype(bf16))                 # [D, D]
    xT = np.ascontiguousarray(x.T).astype(bf16)                 # [D, B]

    in_maps = []
    for c in range(N_CORES):
        sl = slice(c * BS, (c + 1) * BS)
        # chunk-major: xa[p, mb, k, j] = xT[k*128 + p, c*BS + mb*MB + j]
        xa_c = np.ascontiguousarray(
            xT[:, sl].reshape(NK, 128, NMB, MB).transpose(1, 2, 0, 3)
        ).reshape(128, NMB * NK * MB)
        # lga[p, mb, mt, o] = Lg[c*BS + (mb*4+mt)*128 + p, o]
        lga_c = np.ascontiguousarray(
            Lg[sl].reshape(NMB, NMT, 128, D).transpose(2, 0, 1, 3)
        ).reshape(128, NMB * NMT * D)
        in_maps.append({"xa": xa_c, "wa": wa, "lga": lga_c})
    return in_maps


def kernel(x, W, A_table, B_table, domain_id, _trace=False):
    in_maps = _prepare(x, W, A_table, B_table, domain_id)
    nc = _build()
    res = bass_utils.run_bass_kernel_spmd(
        nc, in_maps, core_ids=list(range(N_CORES)), trace=_trace
    )
    out = np.concatenate(
        [np.asarray(res.results[c]["out"]).astype(np.float32) for c in range(N_CORES)],
        axis=0,
    )
    if _trace:
        kernel.last_results = res
    return out
